# revision 2
# baseline (speedup 1.0000x reference)
"""HeteroClassifier GNN kernel for 8 TRN2 NeuronCores (Bass/Tile).

Sharding: L1 edges by dst node-range (owner core aggregates its nodes);
L2 edges by src node-range (gather tables stay core-local); per-core
[B,2] partial outputs are summed to unshard. Host does structure-only
prep (CSR-style degree counts, edge grouping by destination, padded
window layout, index maps). All value compute runs on the NeuronCores:
feat scaling, indirect-DMA gathers (128 idx/instruction), masked
messages, dense windowed segment-reductions (no scatters), FMA matmuls,
relu, pooling.
"""

import numpy as np

import concourse.bass as bass
import concourse.bacc as bacc
import concourse.mybir as mybir
import concourse.tile as tile
from concourse import bass_utils

LAST_EXEC_NS = -1
N = 200000
R = 4
E = 1000000
B = 1024
NCORES = 8
P = 128


def _sizes():
    nb = N // NCORES
    nwin1 = (nb + P - 1) // P
    nwin2 = (B + P - 1) // P
    return nb, nwin1, nwin2


def _plan_and_pack(keys_by_core, nitems, nwin, src_by_core, val_by_core):
    """keys: per-core local group key per edge (dst-local or graph id).
    Returns (ls, ni, per-core list of (idx [P,ni] i32, val [P,ni] f32,
    order [nitems])). Vectorized."""
    orders, cnts_o, edata = [], [], []
    for c in range(NCORES):
        keys = keys_by_core[c]
        cnts = np.bincount(keys, minlength=nitems)
        order = np.argsort(-cnts, kind="stable")
        rp = np.empty(nitems, dtype=np.int64)
        rp[order] = np.arange(nitems)
        six = np.argsort(keys, kind="stable")
        ks = keys[six]
        starts = np.searchsorted(ks, np.arange(nitems))
        j_in_grp = np.arange(len(ks)) - starts[ks]
        orders.append(order)
        cnts_o.append(cnts[order])
        edata.append((six, ks, j_in_grp, rp))
    npad = nwin * P
    ls = []
    for k in range(nwin):
        m = 1
        for c in range(NCORES):
            seg = cnts_o[c][k * P:(k + 1) * P]
            if len(seg):
                m = max(m, int(seg.max()))
        ls.append(m)
    ni = int(np.sum(ls))
    wbase = np.concatenate([[0], np.cumsum(ls)]).astype(np.int64)
    packed = []
    for c in range(NCORES):
        six, ks, j_in_grp, rp = edata[c]
        rank = rp[ks]
        lane = rank % P
        win = rank // P
        col = wbase[win] + j_in_grp
        idx = np.zeros((P, ni), dtype=np.int32)
        val = np.zeros((P, ni), dtype=np.float32)
        idx[lane, col] = src_by_core[c][six]
        val[lane, col] = val_by_core[c][six]
        packed.append((idx, val, orders[c]))
    return ls, ni, packed


def _rank_arrays(order, values, nitems, nwin):
    """values[nitems] -> [P, nwin] in rank order (rank i=(k*P+p) -> order[i])."""
    out = np.zeros((P, nwin), dtype=np.float32)
    npad = nwin * P
    v = np.zeros(npad, dtype=np.float32)
    v[:nitems] = values[order]
    out[:, :] = v.reshape(nwin, P).T
    return out


def _natural_pos(order, present, nitems, nwin):
    """natural item n=(k*P+p) -> rank row in table, or zero-row npad."""
    npad = nwin * P
    rp = np.empty(nitems, dtype=np.int64)
    rp[order] = np.arange(nitems)
    pos = np.full(npad, npad, dtype=np.int32)
    idxs = np.arange(nitems)
    pos[idxs[present]] = rp[present].astype(np.int32)
    return pos.reshape(nwin, P).T.copy()  # [P, nwin]


def _prep(feat, src, dst, ew, graph_ids):
    nb, nwin1, nwin2 = _sizes()
    src = np.asarray(src); dst = np.asarray(dst)
    ew = np.asarray(ew); gid = np.asarray(graph_ids)

    od = np.stack([np.bincount(src[r], minlength=N) for r in range(R)])
    idg = np.stack([np.bincount(dst[r], minlength=N) for r in range(R)])
    cnt = np.bincount(gid, minlength=B)
    ods = (1.0 / np.sqrt(np.clip(od, 1, None))).astype(np.float32)
    ids = (1.0 / np.sqrt(np.clip(idg, 1, None))).astype(np.float32)
    qn = (ids / np.clip(cnt, 1, None)[gid][None, :]).astype(np.float32)

    meta = {"L1": [], "L2": []}
    per_core = [dict() for _ in range(NCORES)]

    for r in range(R):
        core_of = dst[r] // nb
        keys, srcs, vals = [], [], []
        for c in range(NCORES):
            m = core_of == c
            keys.append((dst[r][m] - c * nb).astype(np.int64))
            srcs.append(src[r][m])
            vals.append(ew[r][m])
        ls, ni, packed = _plan_and_pack(keys, nb, nwin1, srcs, vals)
        meta["L1"].append({"ls": ls, "ni": ni})
        for c in range(NCORES):
            idx, val, order = packed[c]
            per_core[c][f"l1idx_{r}"] = idx
            per_core[c][f"l1ew_{r}"] = val
            lo = c * nb
            per_core[c][f"l1rid_{r}"] = _rank_arrays(order, ids[r, lo:lo + nb], nb, nwin1)
            degl = np.bincount(keys[c], minlength=nb)
            per_core[c][f"xpos_{r}"] = _natural_pos(order, degl > 0, nb, nwin1)
            on = np.zeros(nwin1 * P, dtype=np.float32)
            on[:nb] = ods[r, lo:lo + nb]
            per_core[c][f"odsl_{r}"] = on.reshape(nwin1, P).T.copy()

    for r in range(R):
        core_of = src[r] // nb
        keys, srcs, vals = [], [], []
        cnts_local = []
        for c in range(NCORES):
            m = core_of == c
            d = dst[r][m]
            keys.append(gid[d].astype(np.int64))
            srcs.append((src[r][m] - c * nb).astype(np.int32))
            vals.append(qn[r][d])
            cnts_local.append(np.bincount(gid[d], minlength=B))
        ls, ni, packed = _plan_and_pack(keys, B, nwin2, srcs, vals)
        meta["L2"].append({"ls": ls, "ni": ni})
        for c in range(NCORES):
            idx, val, order = packed[c]
            per_core[c][f"l2idx_{r}"] = idx
            per_core[c][f"l2qe_{r}"] = val
            per_core[c][f"gpos_{r}"] = _natural_pos(order, cnts_local[c] > 0, B, nwin2)

    for c in range(NCORES):
        for r in range(R):
            per_core[c][f"ods2_{r}"] = np.repeat(ods[r], 2).astype(np.float32)
    return per_core, meta


def _build_program(meta):
    nb, nwin1, nwin2 = _sizes()
    nc = bacc.Bacc("TRN2", target_bir_lowering=False, debug=False,
                   num_devices=NCORES)
    f32, i32 = mybir.dt.float32, mybir.dt.int32
    AL = mybir.AluOpType

    feat = nc.dram_tensor("feat", [N, 2], f32, kind="ExternalInput").ap()
    w1b = nc.dram_tensor("w1b", [P, R * 2 * 16], f32, kind="ExternalInput").ap()
    b1b = nc.dram_tensor("b1b", [P, R * 16], f32, kind="ExternalInput").ap()
    W2 = nc.dram_tensor("W2", [R, 16, 16], f32, kind="ExternalInput").ap()
    b2b = nc.dram_tensor("b2b", [P, R * 16], f32, kind="ExternalInput").ap()
    Wc = nc.dram_tensor("Wc", [16, 2], f32, kind="ExternalInput").ap()
    bc = nc.dram_tensor("bc", [2], f32, kind="ExternalInput").ap()
    ins = {}
    for r in range(R):
        ni1, ni2 = meta["L1"][r]["ni"], meta["L2"][r]["ni"]
        for nm, shp, dt in (
            (f"l1idx_{r}", [P, ni1], i32), (f"l1ew_{r}", [P, ni1], f32),
            (f"l1rid_{r}", [P, nwin1], f32), (f"xpos_{r}", [P, nwin1], i32),
            (f"odsl_{r}", [P, nwin1], f32),
            (f"l2idx_{r}", [P, ni2], i32), (f"l2qe_{r}", [P, ni2], f32),
            (f"gpos_{r}", [P, nwin2], i32), (f"ods2_{r}", [N * 2], f32),
        ):
            ins[nm] = nc.dram_tensor(nm, shp, dt, kind="ExternalInput").ap()
    hs = [nc.dram_tensor(f"hs_{r}", [N, 2], f32, kind="Internal").ap() for r in range(R)]
    aggR = [nc.dram_tensor(f"aggR_{r}", [(nwin1 + 1) * P, 2], f32, kind="Internal").ap() for r in range(R)]
    gtab = [nc.dram_tensor(f"g_{r}", [nwin1 * P, 2], f32, kind="Internal").ap() for r in range(R)]
    prank = [nc.dram_tensor(f"prank_{r}", [(nwin2 + 1) * P, 2], f32, kind="Internal").ap() for r in range(R)]
    out_part = nc.dram_tensor("out_part", [B, 2], f32, kind="ExternalOutput").ap()
    bias_out = nc.dram_tensor("bias_out", [1, 2], f32, kind="ExternalOutput").ap()
    import os
    DBG = os.environ.get("K_DEBUG") == "1"
    if DBG:
        ni1d = meta["L1"][0]["ni"]
        dbg_hs = nc.dram_tensor("dbg_hs", [N, 2], f32, kind="ExternalOutput").ap()
        dbg_ga = nc.dram_tensor("dbg_ga", [P, ni1d * 2], f32, kind="ExternalOutput").ap()
        dbg_agg = nc.dram_tensor("dbg_agg", [P, nwin1 * 2], f32, kind="ExternalOutput").ap()
        dbg_x = nc.dram_tensor("dbg_x", [P, nwin1 * 2 * R], f32, kind="ExternalOutput").ap()
        dbg_h1 = nc.dram_tensor("dbg_h1", [P, nwin1 * 16], f32, kind="ExternalOutput").ap()
        dbg_g = nc.dram_tensor("dbg_g", [P, nwin1 * 2 * R], f32, kind="ExternalOutput").ap()

    def reduce_windows(ga, out_t, ls, nwin):
        col = 0
        k = 0
        while k < nwin:
            k2 = k
            while k2 < nwin and ls[k2] == ls[k]:
                k2 += 1
            lk, nk = ls[k], k2 - k
            seg = ga[:, col:col + nk * lk, :].rearrange(
                "p (n l) c -> p n c l", l=lk)
            nc.vector.tensor_reduce(out=out_t[:, k:k2, :], in_=seg,
                                    op=AL.add, axis=mybir.AxisListType.X)
            col += nk * lk
            k = k2

    with tile.TileContext(nc) as tc:
        with (tc.tile_pool(name="sbuf", bufs=1) as pool,
              tc.tile_pool(name="psum", bufs=2, space="PSUM") as psum):
            # phase 0: hs tables
            FLAT = (N * 2) // P
            featf = feat.rearrange("n c -> (n c)").rearrange("(p f) -> p f", p=P)
            ft = pool.tile([P, FLAT], f32, name="ft")
            nc.sync.dma_start(out=ft[:], in_=featf)
            for r in range(R):
                ot = pool.tile([P, FLAT], f32, name=f"odst_{r}", tag="odst")
                nc.sync.dma_start(out=ot[:], in_=ins[f"ods2_{r}"].rearrange("(p f) -> p f", p=P))
                hst = pool.tile([P, FLAT], f32, name=f"hst_{r}", tag="hst")
                nc.vector.tensor_tensor(out=hst[:], in0=ft[:], in1=ot[:], op=AL.mult)
                nc.sync.dma_start(
                    out=hs[r].rearrange("n c -> (n c)").rearrange("(p f) -> p f", p=P),
                    in_=hst[:])
            zt = pool.tile([P, 2], f32, name="zt")
            nc.vector.memset(zt[:], 0.0)
            for r in range(R):
                nc.sync.dma_start(out=aggR[r][nwin1 * P:, :], in_=zt[:])
                nc.sync.dma_start(out=prank[r][nwin2 * P:, :], in_=zt[:])

            # phase 1: L1
            for r in range(R):
                ni1, ls = meta["L1"][r]["ni"], meta["L1"][r]["ls"]
                idx_t = pool.tile([P, ni1], i32, name=f"i1_{r}", tag="i1")
                nc.sync.dma_start(out=idx_t[:], in_=ins[f"l1idx_{r}"][:])
                ew_t = pool.tile([P, ni1], f32, name=f"e1_{r}", tag="e1")
                nc.sync.dma_start(out=ew_t[:], in_=ins[f"l1ew_{r}"][:])
                ga = pool.tile([P, ni1, 2], f32, name=f"ga1_{r}", tag="ga1")
                for i in range(ni1):
                    nc.gpsimd.indirect_dma_start(
                        out=ga[:, i, :], out_offset=None, in_=hs[r][:],
                        in_offset=bass.IndirectOffsetOnAxis(ap=idx_t[:, i:i + 1], axis=0))
                nc.vector.tensor_tensor(
                    out=ga[:, :, :], in0=ga[:, :, :],
                    in1=ew_t[:, :, None].to_broadcast([P, ni1, 2]), op=AL.mult)
                if DBG and r == 0:
                    nc.sync.dma_start(out=dbg_ga[:, :], in_=ga[:].rearrange("p a b -> p (a b)"))
                agg_t = pool.tile([P, nwin1, 2], f32, name=f"ag1_{r}", tag="ag1")
                reduce_windows(ga, agg_t, ls, nwin1)
                rid_t = pool.tile([P, nwin1], f32, name=f"rid_{r}", tag="rid")
                nc.sync.dma_start(out=rid_t[:], in_=ins[f"l1rid_{r}"][:])
                nc.vector.tensor_tensor(
                    out=agg_t[:, :, :], in0=agg_t[:, :, :],
                    in1=rid_t[:, :, None].to_broadcast([P, nwin1, 2]), op=AL.mult)
                if DBG and r == 0:
                    nc.sync.dma_start(out=dbg_agg[:, :], in_=agg_t[:].rearrange("p a b -> p (a b)"))
                nc.sync.dma_start(
                    out=aggR[r][:nwin1 * P, :].rearrange("(k p) c -> p k c", p=P),
                    in_=agg_t[:, :, :])

            # phase 2: realign + h1 + g tables
            x_t = pool.tile([P, nwin1, 2 * R], f32, name="x_t")
            for r in range(R):
                xp_t = pool.tile([P, nwin1], i32, name=f"xp_{r}", tag="xp")
                nc.sync.dma_start(out=xp_t[:], in_=ins[f"xpos_{r}"][:])
                for k in range(nwin1):
                    nc.gpsimd.indirect_dma_start(
                        out=x_t[:, k, 2 * r:2 * r + 2], out_offset=None,
                        in_=aggR[r][:],
                        in_offset=bass.IndirectOffsetOnAxis(ap=xp_t[:, k:k + 1], axis=0))
            if DBG:
                nc.sync.dma_start(out=dbg_x[:, :], in_=x_t[:].rearrange("p a b -> p (a b)"))
                hs0f = hs[0].rearrange("n c -> (n c)").rearrange("(p f) -> p f", p=P)
                hsdbg = pool.tile([P, (N * 2) // P], f32, name="hsdbg")
                nc.sync.dma_start(out=hsdbg[:], in_=hs0f)
                nc.sync.dma_start(
                    out=dbg_hs.rearrange("n c -> (n c)").rearrange("(p f) -> p f", p=P),
                    in_=hsdbg[:])
            w1_sb = pool.tile([P, R * 2 * 16], f32, name="w1_sb")
            nc.sync.dma_start(out=w1_sb[:], in_=w1b[:, :])
            b1all = pool.tile([P, R * 16], f32, name="b1all")
            nc.sync.dma_start(out=b1all[:], in_=b1b[:, :])
            b1s = pool.tile([P, 16], f32, name="b1s")
            nc.vector.tensor_reduce(
                out=b1s[:], in_=b1all[:].rearrange("p (r f) -> p f r", r=R),
                op=AL.add, axis=mybir.AxisListType.X)
            h1_t = pool.tile([P, nwin1, 16], f32, name="h1_t")
            tmp1 = pool.tile([P, nwin1, 1], f32, name="tmp1")
            tmp2 = pool.tile([P, nwin1, 1], f32, name="tmp2")
            for f in range(16):
                for c in range(2 * R):
                    w_ap = w1_sb[:, c * 16 + f:c * 16 + f + 1][:, :, None].to_broadcast([P, nwin1, 1])
                    if c == 0:
                        nc.vector.tensor_tensor(out=tmp1[:, :, :], in0=x_t[:, :, 0:1],
                                                in1=w_ap, op=AL.mult)
                    else:
                        nc.vector.tensor_tensor(out=tmp2[:, :, :], in0=x_t[:, :, c:c + 1],
                                                in1=w_ap, op=AL.mult)
                        nc.vector.tensor_tensor(out=tmp1[:, :, :], in0=tmp1[:, :, :],
                                                in1=tmp2[:, :, :], op=AL.add)
                b_ap = b1s[:, f:f + 1][:, :, None].to_broadcast([P, nwin1, 1])
                nc.vector.tensor_tensor(out=h1_t[:, :, f:f + 1], in0=tmp1[:, :, :],
                                        in1=b_ap, op=AL.add)
            nc.vector.tensor_scalar_max(h1_t[:, :, :], h1_t[:, :, :], 0.0)

            if DBG:
                nc.sync.dma_start(out=dbg_h1[:, :], in_=h1_t[:].rearrange("p a b -> p (a b)"))
            wc_sb = pool.tile([16, 2], f32, name="wc_sb")
            nc.sync.dma_start(out=wc_sb[:], in_=Wc[:, :])
            m_sb = pool.tile([1, R * 32], f32, name="m_sb")
            ones_sb = pool.tile([1, P], f32, name="ones_sb")
            nc.vector.memset(ones_sb[:], 1.0)
            for r in range(R):
                w2_sb = pool.tile([16, 16], f32, name=f"w2_{r}", tag="w2")
                nc.sync.dma_start(out=w2_sb[:], in_=W2[r, :, :].rearrange("a b -> b a"))
                m_ps = psum.tile([16, 2], f32, name=f"mps_{r}", tag="mps")
                nc.tensor.matmul(out=m_ps[:], lhsT=w2_sb[:], rhs=wc_sb[:],
                                 start=True, stop=True)
                mt = pool.tile([16, 2], f32, name=f"mt_{r}", tag="mt")
                nc.vector.tensor_copy(out=mt[:], in_=m_ps[:])
                md = nc.dram_tensor(f"m_dram_{r}", [16, 2], f32, kind="Internal").ap()
                nc.sync.dma_start(out=md[:, :], in_=mt[:])
                nc.sync.dma_start(out=m_sb[:, r * 32:(r + 1) * 32],
                                  in_=md.rearrange("f c -> (f c)")[None, :])
            mb_ps = psum.tile([P, R * 32], f32, name="mb_ps")
            nc.tensor.matmul(out=mb_ps[:], lhsT=ones_sb[:], rhs=m_sb[:],
                             start=True, stop=True)
            mb = pool.tile([P, R * 32], f32, name="mb")
            nc.vector.tensor_copy(out=mb[:], in_=mb_ps[:])
            g_t = pool.tile([P, nwin1, 2 * R], f32, name="g_t")
            for r in range(R):
                for cch in range(2):
                    for f in range(16):
                        w_ap = mb[:, r * 32 + f * 2 + cch:r * 32 + f * 2 + cch + 1][:, :, None] \
                            .to_broadcast([P, nwin1, 1])
                        if f == 0:
                            nc.vector.tensor_tensor(out=tmp1[:, :, :], in0=h1_t[:, :, 0:1],
                                                    in1=w_ap, op=AL.mult)
                        else:
                            nc.vector.tensor_tensor(out=tmp2[:, :, :], in0=h1_t[:, :, f:f + 1],
                                                    in1=w_ap, op=AL.mult)
                            nc.vector.tensor_tensor(out=tmp1[:, :, :], in0=tmp1[:, :, :],
                                                    in1=tmp2[:, :, :], op=AL.add)
                    nc.vector.tensor_copy(
                        out=g_t[:, :, 2 * r + cch:2 * r + cch + 1], in_=tmp1[:, :, :])
            for r in range(R):
                ol_t = pool.tile([P, nwin1], f32, name=f"ol_{r}", tag="ol")
                nc.sync.dma_start(out=ol_t[:], in_=ins[f"odsl_{r}"][:])
                nc.vector.tensor_tensor(
                    out=g_t[:, :, 2 * r:2 * r + 2], in0=g_t[:, :, 2 * r:2 * r + 2],
                    in1=ol_t[:, :, None].to_broadcast([P, nwin1, 2]), op=AL.mult)
                nc.sync.dma_start(
                    out=gtab[r].rearrange("(k p) c -> p k c", p=P),
                    in_=g_t[:, :, 2 * r:2 * r + 2])

            if DBG:
                nc.sync.dma_start(out=dbg_g[:, :], in_=g_t[:].rearrange("p a b -> p (a b)"))
            # phase 3: L2
            for r in range(R):
                ni2, ls = meta["L2"][r]["ni"], meta["L2"][r]["ls"]
                idx_t = pool.tile([P, ni2], i32, name=f"i2_{r}", tag="i2")
                nc.sync.dma_start(out=idx_t[:], in_=ins[f"l2idx_{r}"][:])
                qe_t = pool.tile([P, ni2], f32, name=f"q2_{r}", tag="q2")
                nc.sync.dma_start(out=qe_t[:], in_=ins[f"l2qe_{r}"][:])
                ga = pool.tile([P, ni2, 2], f32, name=f"ga2_{r}", tag="ga2")
                for i in range(ni2):
                    nc.gpsimd.indirect_dma_start(
                        out=ga[:, i, :], out_offset=None, in_=gtab[r][:],
                        in_offset=bass.IndirectOffsetOnAxis(ap=idx_t[:, i:i + 1], axis=0))
                nc.vector.tensor_tensor(
                    out=ga[:, :, :], in0=ga[:, :, :],
                    in1=qe_t[:, :, None].to_broadcast([P, ni2, 2]), op=AL.mult)
                pr_t = pool.tile([P, nwin2, 2], f32, name=f"pr_{r}", tag="pr")
                reduce_windows(ga, pr_t, ls, nwin2)
                nc.sync.dma_start(
                    out=prank[r][:nwin2 * P, :].rearrange("(k p) c -> p k c", p=P),
                    in_=pr_t[:, :, :])

            # phase 4: realign graphs, sum relations, bias, output
            osum = pool.tile([P, nwin2, 2], f32, name="osum")
            for r in range(R):
                gp_t = pool.tile([P, nwin2], i32, name=f"gp_{r}", tag="gp")
                nc.sync.dma_start(out=gp_t[:], in_=ins[f"gpos_{r}"][:])
                gr = pool.tile([P, nwin2, 2], f32, name=f"gr_{r}", tag="gr")
                for k in range(nwin2):
                    nc.gpsimd.indirect_dma_start(
                        out=gr[:, k, :], out_offset=None, in_=prank[r][:],
                        in_offset=bass.IndirectOffsetOnAxis(ap=gp_t[:, k:k + 1], axis=0))
                if r == 0:
                    nc.vector.tensor_copy(out=osum[:, :, :], in_=gr[:, :, :])
                else:
                    nc.vector.tensor_add(out=osum[:, :, :], in0=osum[:, :, :],
                                         in1=gr[:, :, :])
            nc.sync.dma_start(
                out=out_part.rearrange("(k p) c -> p k c", p=P),
                in_=osum[:, :, :])
            b2all = pool.tile([P, R * 16], f32, name="b2all")
            nc.sync.dma_start(out=b2all[:], in_=b2b[:, :])
            b2s = pool.tile([P, 16], f32, name="b2s")
            nc.vector.tensor_reduce(
                out=b2s[:], in_=b2all[:].rearrange("p (r f) -> p f r", r=R),
                op=AL.add, axis=mybir.AxisListType.X)
            b2d = nc.dram_tensor("b2s_dram", [16], f32, kind="Internal").ap()
            nc.sync.dma_start(out=b2d[None, :], in_=b2s[0:1, :])
            b2col = pool.tile([16, 1], f32, name="b2col")
            nc.sync.dma_start(out=b2col[:], in_=b2d[:, None])
            bo_ps = psum.tile([1, 2], f32, name="bo_ps")
            nc.tensor.matmul(out=bo_ps[:], lhsT=b2col[:], rhs=wc_sb[:],
                             start=True, stop=True)
            bc_sb = pool.tile([1, 2], f32, name="bc_sb")
            nc.sync.dma_start(out=bc_sb[:], in_=bc[None, :])
            bo_sb = pool.tile([1, 2], f32, name="bo_sb")
            nc.vector.tensor_add(out=bo_sb[:], in0=bo_ps[:], in1=bc_sb[:])
            nc.sync.dma_start(out=bias_out[:, :], in_=bo_sb[:])
    nc.compile()
    return nc


def kernel(feat, src, dst, ew, graph_ids, W1, b1, W2, b2, Wc, bc):
    per_core, meta = _prep(feat, src, dst, ew, graph_ids)
    nc = _build_program(meta)
    w1f = np.ascontiguousarray(W1, dtype=np.float32).reshape(-1)
    b1f = np.ascontiguousarray(b1, dtype=np.float32).reshape(-1)
    b2f = np.ascontiguousarray(b2, dtype=np.float32).reshape(-1)
    common = {
        "feat": np.ascontiguousarray(feat, dtype=np.float32),
        "w1b": np.tile(w1f, (P, 1)),
        "b1b": np.tile(b1f, (P, 1)),
        "W2": np.ascontiguousarray(W2, dtype=np.float32),
        "b2b": np.tile(b2f, (P, 1)),
        "Wc": np.ascontiguousarray(Wc, dtype=np.float32),
        "bc": np.ascontiguousarray(bc, dtype=np.float32),
    }
    in_maps = [{**common, **per_core[c]} for c in range(NCORES)]
    import os
    import time as _t
    _t0 = _t.perf_counter()
    kw = {}
    if os.environ.get("K_TRACE") == "1":
        kw = {"trace": True, "tmpdir": os.environ.get("K_TRACE_DIR") or None}
    res = bass_utils.run_bass_kernel_spmd(nc, in_maps, core_ids=list(range(NCORES)),
                                          **kw)
    global LAST_EXEC_NS
    LAST_EXEC_NS = int((_t.perf_counter() - _t0) * 1e9)
    if getattr(res, "exec_time_ns", None):
        LAST_EXEC_NS = int(res.exec_time_ns)
        if res.instructions_and_trace is not None:
            print("trace path:", res.instructions_and_trace[1])
    out = np.zeros((B, 2), dtype=np.float32)
    for c in range(NCORES):
        out += res.results[c]["out_part"]
    out += res.results[0]["bias_out"][0]
    return out

